# revision 18
# baseline (speedup 1.0000x reference)
"""Trainium2 kernel for the DANet CAM (channel-attention) module.

Per batch b (x: [16, 512, 64, 64] fp32, gamma: [1] fp32):
    q      = x[b].reshape(C, HW)                       # C=512, HW=4096
    energy = q @ q.T                                   # [C, C]
    att    = softmax(max_row(energy) - energy)         # row-wise
           = exp(min_row(energy) - energy) / row_sum(...)
    out[b] = gamma * (att @ q) + x[b]

Sharding: data-parallel over batch -> 2 batches per NeuronCore x 8 cores.
All matmuls (Gram, attention apply, and PE transposes) run in float32r
(full-rate 4-byte PE mode); softmax runs on the ACT/DVE engines.

Structure per batch (PE kept dense):
  - q loaded in 512-col slices (transposes can start after ~1 MB arrives)
  - PE transposes q -> qT (packed [128, 1024] tiles), interleaved with the
    Gram accumulation over 32 chunks into 4 PSUM banks
  - per row-tile i: row-min (DVE) -> exp with accum row-sum (ACT, writes
    f32r) -> 4 PE transposes (attT_i) -> 32 out matmuls; the gamma/row-sum
    normalization is folded into the final residual op
    ot = (psum * gamma/s) + x (single DVE scalar_tensor_tensor);
    x is re-DMAed in fp32 directly into the output staging tile so the
    residual is exact (the f32r q copy is rounded by the DMA engine).
"""

import numpy as np

import concourse.bacc as bacc
import concourse.mybir as mybir
from concourse.bass_utils import run_bass_kernel_spmd
from concourse.masks import make_identity
from concourse.tile import TileContext

P = 128
C = 512
HW = 4096
BPC = 2                # batches per core
NCORES = 8
B = BPC * NCORES       # 16
CT = C // P            # 4 channel tiles
NQT = HW // 256        # 16 packed qT tiles (2 transpose chunks of 128 each)
NS = HW // 512         # 8 column slices
F32 = mybir.dt.float32
F32R = mybir.dt.float32r
AX = mybir.AxisListType
OPT = mybir.AluOpType
ACTF = mybir.ActivationFunctionType


def build(bpc=BPC, qbufs=8, exact=True):
    nc = bacc.Bacc(None, target_bir_lowering=False, debug=False)
    x = nc.dram_tensor("x", [bpc, C, HW], F32R, kind="ExternalInput")
    gam = nc.dram_tensor("gamma", [P, 1], F32, kind="ExternalInput")
    out = nc.dram_tensor("out", [bpc, C, HW], F32, kind="ExternalOutput")

    with TileContext(nc) as tc:
        with (
            tc.tile_pool(name="cpool", bufs=1) as cpool,
            tc.tile_pool(name="qpool", bufs=qbufs) as qpool,
            tc.tile_pool(name="qtpool", bufs=6) as qtpool,
            tc.tile_pool(name="apool", bufs=4) as apool,
            tc.tile_pool(name="atpool", bufs=4) as atpool,
            tc.tile_pool(name="opool", bufs=8) as opool,
            tc.tile_pool(name="spool", bufs=6) as spool,
            tc.tile_pool(name="mmp", bufs=4, space="PSUM") as mmp,
            tc.tile_pool(name="tpp", bufs=2, space="PSUM") as tpp,
        ):
            ident_f = cpool.tile([P, P], F32, name="ident_f")
            make_identity(nc, ident_f[:])
            ident = cpool.tile([P, P], F32R, name="ident")
            nc.vector.tensor_copy(ident[:], ident_f[:])
            g = cpool.tile([P, 1], F32, name="g")
            nc.sync.dma_start(g[:], gam[:, :])

            q_tiles = {}

            def load_q(b, cts):
                """Allocate + DMA q tiles (ns-major within this group)."""
                tiles = q_tiles.setdefault(b, {})
                for ct in cts:
                    tiles[ct] = qpool.tile([P, HW], F32R, name=f"q{b}_{ct}",
                                           tag="q")
                for ns in range(NS):
                    for ct in cts:
                        nc.sync.dma_start(
                            tiles[ct][:, ns * 512:(ns + 1) * 512],
                            x[b, ct * P:(ct + 1) * P, ns * 512:(ns + 1) * 512],
                        )

            load_q(0, range(CT))

            for b in range(bpc):
                q = [q_tiles[b][ct] for ct in range(CT)]

                # prefetch half the next batch early; the rest mid-out-phase
                # (keeps the DMA fabric evenly loaded across the batch)
                if b + 1 < bpc:
                    load_q(b + 1, range(2))

                # ---- qT via PE transposes, interleaved with the Gram
                # accumulation. qt tile k ([128, 1024]) packs transpose
                # chunks 2k (cols 0:512) and 2k+1 (cols 512:1024); chunk n
                # holds q[:, n*128:(n+1)*128].T as [n-in-chunk, c].
                qt = [None] * NQT

                def emit_transpose(k, b=b, q=q, qt=qt):
                    tp = tpp.tile([P, 1024], F32, name=f"tp{b}_{k}", tag="tp")
                    for u in range(2):
                        n = 2 * k + u
                        for ct in range(CT):
                            dst = tp[:, u * 512 + ct * P: u * 512 + (ct + 1) * P]
                            nc.tensor.transpose(
                                dst.bitcast(F32R),
                                q[ct][:, n * P:(n + 1) * P],
                                ident[:],
                            )
                    qtk = qtpool.tile([P, 1024], F32R, name=f"qt{b}_{k}", tag="qt")
                    if k % 2 == 0:
                        nc.vector.tensor_copy(qtk[:], tp[:])
                    else:
                        nc.scalar.copy(qtk[:], tp[:])
                    qt[k] = qtk

                e = [
                    mmp.tile([P, C], F32, name=f"e{b}_{i}", tag="mm")
                    for i in range(CT)
                ]

                def emit_energy(k, b=b, e=e, qt=qt):
                    for u in range(2):
                        first = (k == 0 and u == 0)
                        last = (k == NQT - 1 and u == 1)
                        rhs = qt[k][:, u * 512:(u + 1) * 512]
                        for i in range(CT):
                            lhsT = qt[k][:, u * 512 + i * P: u * 512 + (i + 1) * P]
                            nc.tensor.matmul(
                                e[i][:], lhsT=lhsT, rhs=rhs,
                                start=first, stop=last,
                            )

                emit_transpose(0)
                emit_transpose(1)
                for k in range(NQT):
                    if k + 2 < NQT:
                        emit_transpose(k + 2)
                    emit_energy(k)

                # ---- softmax chains for all row tiles first (keeps the
                # in-order DVE/ACT queues from serializing softmax(i+1)
                # behind tile i's residual adds)
                fs, atTs = [], []
                for i in range(CT):
                    mn = spool.tile([P, 1], F32, name=f"mn{b}_{i}", tag="mn")
                    nc.vector.tensor_reduce(
                        mn[:], e[i][:], axis=AX.X, op=OPT.min
                    )
                    # ea = exp(mn - e) (unnormalized attention row block,
                    # rounded to f32r for the PE), s = row sums
                    ea = apool.tile([P, C], F32R, name=f"att{b}_{i}", tag="att")
                    s = spool.tile([P, 1], F32, name=f"s{b}_{i}", tag="s")
                    nc.scalar.activation(
                        ea[:], e[i][:], ACTF.Exp,
                        bias=mn[:], scale=-1.0, accum_out=s[:],
                    )
                    # f = gamma / s  (folded into the residual add below)
                    rinv = spool.tile([P, 1], F32, name=f"ri{b}_{i}", tag="ri")
                    nc.vector.reciprocal(rinv[:], s[:])
                    f = spool.tile([P, 1], F32, name=f"f{b}_{i}", tag="f")
                    nc.vector.tensor_mul(f[:], rinv[:], g[:])
                    fs.append(f)

                    # attT_i[:, j*128:(j+1)*128] = ea[:, j-block].T
                    tq = tpp.tile([P, 512], F32, name=f"tq{b}_{i}", tag="tp")
                    for j in range(CT):
                        nc.tensor.transpose(
                            tq[:, j * P:(j + 1) * P].bitcast(F32R),
                            ea[:, j * P:(j + 1) * P],
                            ident[:],
                        )
                    atT = atpool.tile([P, C], F32R, name=f"attT{b}_{i}", tag="attT")
                    if i % 2 == 0:
                        nc.scalar.copy(atT[:], tq[:])
                    else:
                        nc.vector.tensor_copy(atT[:], tq[:])
                    atTs.append(atT)

                # ---- out rows: ot = (attT_i.T @ q) * f + x, in 16 groups
                # of [128, 1024]. x-residual DMAs are issued ~6 groups ahead
                # so the fp32 x data is resident when the adds run.
                groups = [(i, h) for i in range(CT) for h in range(4)]
                ots = {}

                def alloc_ot(i, h, b=b, ots=ots):
                    ot = opool.tile([P, 1024], F32, name=f"ot{b}_{i}_{h}",
                                    tag="ot")
                    if exact:
                        nc.sync.dma_start(
                            ot[:],
                            x[b, i * P:(i + 1) * P, h * 1024:(h + 1) * 1024]
                            .bitcast(F32),
                        )
                    ots[(i, h)] = ot

                LOOKAHEAD = 6
                for idx in range(LOOKAHEAD):
                    alloc_ot(*groups[idx])
                for idx, (i, h) in enumerate(groups):
                    if idx + LOOKAHEAD < len(groups):
                        alloc_ot(*groups[idx + LOOKAHEAD])
                    if idx == 8 and b + 1 < bpc:
                        load_q(b + 1, range(2, CT))
                    ot = ots.pop((i, h))
                    f, atT = fs[i], atTs[i]
                    for v in range(2):
                        nn = h * 2 + v
                        op = mmp.tile([P, 512], F32, name=f"op{b}_{i}_{nn}",
                                      tag="mm")
                        for j in range(CT):
                            nc.tensor.matmul(
                                op[:],
                                lhsT=atT[:, j * P:(j + 1) * P],
                                rhs=q[j][:, nn * 512:(nn + 1) * 512],
                                start=(j == 0), stop=(j == CT - 1),
                            )
                        if exact:
                            nc.vector.scalar_tensor_tensor(
                                ot[:, v * 512:(v + 1) * 512],
                                op[:],
                                f[:],
                                ot[:, v * 512:(v + 1) * 512],
                                op0=OPT.mult,
                                op1=OPT.add,
                            )
                        else:
                            nc.vector.scalar_tensor_tensor(
                                ot[:, v * 512:(v + 1) * 512],
                                op[:],
                                f[:],
                                q[i][:, nn * 512:(nn + 1) * 512].bitcast(F32),
                                op0=OPT.mult,
                                op1=OPT.add,
                            )
                    nc.sync.dma_start(
                        out[b, i * P:(i + 1) * P, h * 1024:(h + 1) * 1024],
                        ot[:],
                    )

    nc.compile()
    return nc


def build_v2(bpc=BPC):
    """v2: single fp32 x load per batch (exact residual from SBUF), fp32
    PE transposes, f32r produced only by on-chip rounding copies; the out
    matmul rhs comes from a small round-copy ring (out loop h-major)."""
    nc = bacc.Bacc(None, target_bir_lowering=False, debug=False)
    x = nc.dram_tensor("x", [bpc, C, HW], F32, kind="ExternalInput")
    gam = nc.dram_tensor("gamma", [P, 1], F32, kind="ExternalInput")
    out = nc.dram_tensor("out", [bpc, C, HW], F32, kind="ExternalOutput")

    with TileContext(nc) as tc:
        with (
            tc.tile_pool(name="cpool", bufs=1) as cpool,
            tc.tile_pool(name="xfpool", bufs=6) as xfpool,
            tc.tile_pool(name="qtpool", bufs=6) as qtpool,
            tc.tile_pool(name="qrpool", bufs=8) as qrpool,
            tc.tile_pool(name="apool", bufs=4) as apool,
            tc.tile_pool(name="atpool", bufs=4) as atpool,
            tc.tile_pool(name="opool", bufs=4) as opool,
            tc.tile_pool(name="spool", bufs=6) as spool,
            tc.tile_pool(name="mmp", bufs=4, space="PSUM") as mmp,
            tc.tile_pool(name="tpp", bufs=2, space="PSUM") as tpp,
        ):
            ident_f = cpool.tile([P, P], F32, name="ident_f")
            make_identity(nc, ident_f[:])
            ident = cpool.tile([P, P], F32R, name="ident")
            nc.vector.tensor_copy(ident[:], ident_f[:])
            g = cpool.tile([P, 1], F32, name="g")
            nc.sync.dma_start(g[:], gam[:, :])

            xf_tiles = {}

            def load_xf(b, cts, fine_first=False):
                tiles = xf_tiles.setdefault(b, {})
                for ct in cts:
                    tiles[ct] = xfpool.tile([P, HW], F32, name=f"xf{b}_{ct}",
                                            tag="xf")
                if fine_first:
                    # first transpose chunk needs only cols 0:128 of each
                    # tile: land those first so the PE starts sooner
                    for ct in cts:
                        nc.sync.dma_start(
                            tiles[ct][:, 0:P],
                            x[b, ct * P:(ct + 1) * P, 0:P],
                        )
                for ns in range(NS):
                    for ct in cts:
                        lo = P if (fine_first and ns == 0) else 0
                        nc.sync.dma_start(
                            tiles[ct][:, ns * 512 + lo:(ns + 1) * 512],
                            x[b, ct * P:(ct + 1) * P, ns * 512 + lo:(ns + 1) * 512],
                        )

            load_xf(0, range(CT), fine_first=True)

            for b in range(bpc):
                xf = [xf_tiles[b][ct] for ct in range(CT)]

                if b + 1 < bpc:
                    load_xf(b + 1, range(2))

                # ---- f32r slice ring (qs): rounded copies of xf used as
                # transpose inputs (f32r transposes are 1.5 cyc/row vs 2.0
                # for fp32); the same pool serves the out-phase rhs ring.
                qs = {}

                def load_qs(h, b=b, xf=xf, qs=qs):
                    for ct in range(CT):
                        t = qrpool.tile([P, 1024], F32R, name=f"qs{b}_{h}_{ct}",
                                        tag="qr")
                        if (h + ct) % 2 == 0:
                            nc.vector.tensor_copy(
                                t[:], xf[ct][:, h * 1024:(h + 1) * 1024])
                        else:
                            nc.scalar.copy(
                                t[:], xf[ct][:, h * 1024:(h + 1) * 1024])
                        qs[(h, ct)] = t

                # ---- qT via f32r PE transposes; Gram accumulation in f32r
                qt = [None] * NQT

                def emit_transpose(k, b=b, xf=xf, qs=qs, qt=qt):
                    tp = tpp.tile([P, 1024], F32, name=f"tp{b}_{k}", tag="tp")
                    for u in range(2):
                        n = 2 * k + u
                        h, r = n // 8, n % 8
                        for ct in range(CT):
                            dst = tp[:, u * 512 + ct * P: u * 512 + (ct + 1) * P]
                            if h == 0:
                                # first column group straight from xf (fp32
                                # transpose): avoids the DMA->round-copy
                                # latency chain at batch start
                                nc.tensor.transpose(
                                    dst,
                                    xf[ct][:, n * P:(n + 1) * P],
                                    ident_f[:],
                                )
                            else:
                                nc.tensor.transpose(
                                    dst.bitcast(F32R),
                                    qs[(h, ct)][:, r * P:(r + 1) * P],
                                    ident[:],
                                )
                    qtk = qtpool.tile([P, 1024], F32R, name=f"qt{b}_{k}", tag="qt")
                    if k % 2 == 0:
                        nc.vector.tensor_copy(qtk[:], tp[:])
                    else:
                        nc.scalar.copy(qtk[:], tp[:])
                    qt[k] = qtk

                e = [
                    mmp.tile([P, C], F32, name=f"e{b}_{i}", tag="mm")
                    for i in range(CT)
                ]

                # energy rows i=0,1,2 only from column i*128 on (the
                # Gram matrix is symmetric); row 3 in full (a 128-wide f32r
                # matmul would run at 1/4 rate, so full width is cheaper).
                # Missing blocks (1,0), (2,0), (2,1) are mirrored afterwards.
                ECOL = [0, P, 2 * P, 0]

                def emit_energy(k, b=b, e=e, qt=qt):
                    for u in range(2):
                        first = (k == 0 and u == 0)
                        last = (k == NQT - 1 and u == 1)
                        for i in (3, 0, 1, 2):
                            c0 = ECOL[i]
                            rhs = qt[k][:, u * 512 + c0:(u + 1) * 512]
                            lhsT = qt[k][:, u * 512 + i * P: u * 512 + (i + 1) * P]
                            nc.tensor.matmul(
                                e[i][:, c0:], lhsT=lhsT, rhs=rhs,
                                start=first, stop=last,
                            )

                load_qs(1)
                emit_transpose(0)
                emit_transpose(1)
                for k in range(NQT):
                    if k % 4 == 0 and (k // 4 + 2) < 4:
                        load_qs(k // 4 + 2)
                    if k + 2 < NQT:
                        emit_transpose(k + 2)
                    emit_energy(k)

                # ---- softmax chains for all row tiles; row 3 first (it
                # needs no mirrored blocks, so its chain starts immediately
                # after the last Gram matmul)
                fs, atTs = {}, {}

                def emit_mirrors(b=b, e=e):
                    mst = apool.tile([P, 3 * P], F32R, name=f"mst{b}", tag="mst")
                    for m, (di, dj) in enumerate([(1, 0), (2, 0), (2, 1)]):
                        nc.scalar.copy(
                            mst[:, m * P:(m + 1) * P],
                            e[dj][:, di * P:(di + 1) * P],
                        )
                        nc.tensor.transpose(
                            e[di][:, dj * P:(dj + 1) * P].bitcast(F32R),
                            mst[:, m * P:(m + 1) * P],
                            ident[:],
                        )

                for i in (3, 0, 1, 2):
                    mn = spool.tile([P, 1], F32, name=f"mn{b}_{i}", tag="mn")
                    nc.vector.tensor_reduce(
                        mn[:], e[i][:], axis=AX.X, op=OPT.min
                    )
                    ea = apool.tile([P, C], F32R, name=f"att{b}_{i}", tag="att")
                    s = spool.tile([P, 1], F32, name=f"s{b}_{i}", tag="s")
                    nc.scalar.activation(
                        ea[:], e[i][:], ACTF.Exp,
                        bias=mn[:], scale=-1.0, accum_out=s[:],
                    )
                    rinv = spool.tile([P, 1], F32, name=f"ri{b}_{i}", tag="ri")
                    nc.vector.reciprocal(rinv[:], s[:])
                    f = spool.tile([P, 1], F32, name=f"f{b}_{i}", tag="f")
                    nc.vector.tensor_mul(f[:], rinv[:], g[:])
                    fs[i] = f

                    tq = tpp.tile([P, 512], F32, name=f"tq{b}_{i}", tag="tp")
                    for j in range(CT):
                        nc.tensor.transpose(
                            tq[:, j * P:(j + 1) * P].bitcast(F32R),
                            ea[:, j * P:(j + 1) * P],
                            ident[:],
                        )
                    atT = atpool.tile([P, C], F32R, name=f"attT{b}_{i}", tag="attT")
                    if i % 2 == 0:
                        nc.scalar.copy(atT[:], tq[:])
                    else:
                        nc.vector.tensor_copy(atT[:], tq[:])
                    atTs[i] = atT
                    if i == 3:
                        emit_mirrors()

                # ---- out phase, h-major: for each column group h (1024 wide)
                # round-copy the 4 rhs slices xf[j][:, h] -> f32r ring, then
                # compute the 4 row tiles i.
                qr = {}

                def load_qr(h, b=b, xf=xf, qr=qr):
                    for j in range(CT):
                        t = qrpool.tile([P, 1024], F32R, name=f"qr{b}_{h}_{j}",
                                        tag="qr")
                        if j % 2 == 0:
                            nc.vector.tensor_copy(
                                t[:], xf[j][:, h * 1024:(h + 1) * 1024])
                        else:
                            nc.scalar.copy(
                                t[:], xf[j][:, h * 1024:(h + 1) * 1024])
                        qr[(h, j)] = t

                load_qr(0)
                for h in range(4):
                    if h + 1 < 4:
                        load_qr(h + 1)
                    if h == 2 and b + 1 < bpc:
                        load_xf(b + 1, range(2, CT))
                    for i in (3, 0, 1, 2):
                        f, atT = fs[i], atTs[i]
                        ot = opool.tile([P, 1024], F32, name=f"ot{b}_{i}_{h}",
                                        tag="ot")
                        for v in range(2):
                            nn = h * 2 + v
                            op = mmp.tile([P, 512], F32, name=f"op{b}_{i}_{nn}",
                                          tag="mm")
                            for j in range(CT):
                                nc.tensor.matmul(
                                    op[:],
                                    lhsT=atT[:, j * P:(j + 1) * P],
                                    rhs=qr[(h, j)][:, v * 512:(v + 1) * 512],
                                    start=(j == 0), stop=(j == CT - 1),
                                )
                            nc.vector.scalar_tensor_tensor(
                                ot[:, v * 512:(v + 1) * 512],
                                op[:],
                                f[:],
                                xf[i][:, nn * 512:(nn + 1) * 512],
                                op0=OPT.mult,
                                op1=OPT.add,
                            )
                        nc.sync.dma_start(
                            out[b, i * P:(i + 1) * P, h * 1024:(h + 1) * 1024],
                            ot[:],
                        )
                    for j in range(CT):
                        qr.pop((h, j))
    nc.compile()
    return nc


_NC_CACHE = None


def _get_nc():
    global _NC_CACHE
    if _NC_CACHE is None:
        _NC_CACHE = build_v2()
    return _NC_CACHE


def run(x, gamma, trace=False):
    """x: [16, 512, 64, 64] fp32; gamma: [1] fp32. Returns (y, results)."""
    x = np.ascontiguousarray(np.asarray(x, dtype=np.float32)).reshape(B, C, HW)
    gval = np.float32(np.asarray(gamma, dtype=np.float32).reshape(-1)[0])
    gbc = np.full((P, 1), gval, dtype=np.float32)
    nc = _get_nc()
    in_maps = [
        {"x": np.ascontiguousarray(x[i * BPC:(i + 1) * BPC]), "gamma": gbc}
        for i in range(NCORES)
    ]
    results = run_bass_kernel_spmd(
        nc, in_maps, core_ids=list(range(NCORES)), trace=trace
    )
    y = np.concatenate([r["out"] for r in results.results], axis=0)
    return y.reshape(B, C, 64, 64), results


def kernel(x, gamma):
    y, _ = run(x, gamma)
    return y


if __name__ == "__main__":
    rng = np.random.default_rng(0)
    x = rng.standard_normal((B, C, 64, 64)).astype(np.float32)
    gamma = np.zeros((1,), dtype=np.float32)
    y, _ = run(x, gamma)
    print("gamma=0 exact:", np.array_equal(y, x))


# revision 20
# speedup vs baseline: 1.0197x; 1.0197x over previous
"""Trainium2 kernel for the DANet CAM (channel-attention) module.

Per batch b (x: [16, 512, 64, 64] fp32, gamma: [1] fp32):
    q      = x[b].reshape(C, HW)                       # C=512, HW=4096
    energy = q @ q.T                                   # [C, C]
    att    = softmax(max_row(energy) - energy)         # row-wise
           = exp(min_row(energy) - energy) / row_sum(...)
    out[b] = gamma * (att @ q) + x[b]

Sharding: data-parallel over batch -> 2 batches per NeuronCore x 8 cores.
All matmuls (Gram, attention apply, and PE transposes) run in float32r
(full-rate 4-byte PE mode); softmax runs on the ACT/DVE engines.

Structure per batch (PE kept dense):
  - q loaded in 512-col slices (transposes can start after ~1 MB arrives)
  - PE transposes q -> qT (packed [128, 1024] tiles), interleaved with the
    Gram accumulation over 32 chunks into 4 PSUM banks
  - per row-tile i: row-min (DVE) -> exp with accum row-sum (ACT, writes
    f32r) -> 4 PE transposes (attT_i) -> 32 out matmuls; the gamma/row-sum
    normalization is folded into the final residual op
    ot = (psum * gamma/s) + x (single DVE scalar_tensor_tensor);
    x is re-DMAed in fp32 directly into the output staging tile so the
    residual is exact (the f32r q copy is rounded by the DMA engine).
"""

import numpy as np

import concourse.bacc as bacc
import concourse.mybir as mybir
from concourse.bass_utils import run_bass_kernel_spmd
from concourse.masks import make_identity
from concourse.tile import TileContext

P = 128
C = 512
HW = 4096
BPC = 2                # batches per core
NCORES = 8
B = BPC * NCORES       # 16
CT = C // P            # 4 channel tiles
NQT = HW // 256        # 16 packed qT tiles (2 transpose chunks of 128 each)
NS = HW // 512         # 8 column slices
F32 = mybir.dt.float32
F32R = mybir.dt.float32r
AX = mybir.AxisListType
OPT = mybir.AluOpType
ACTF = mybir.ActivationFunctionType


def build(bpc=BPC, qbufs=8, exact=True):
    nc = bacc.Bacc(None, target_bir_lowering=False, debug=False)
    x = nc.dram_tensor("x", [bpc, C, HW], F32R, kind="ExternalInput")
    gam = nc.dram_tensor("gamma", [P, 1], F32, kind="ExternalInput")
    out = nc.dram_tensor("out", [bpc, C, HW], F32, kind="ExternalOutput")

    with TileContext(nc) as tc:
        with (
            tc.tile_pool(name="cpool", bufs=1) as cpool,
            tc.tile_pool(name="qpool", bufs=qbufs) as qpool,
            tc.tile_pool(name="qtpool", bufs=6) as qtpool,
            tc.tile_pool(name="apool", bufs=4) as apool,
            tc.tile_pool(name="atpool", bufs=4) as atpool,
            tc.tile_pool(name="opool", bufs=8) as opool,
            tc.tile_pool(name="spool", bufs=6) as spool,
            tc.tile_pool(name="mmp", bufs=4, space="PSUM") as mmp,
            tc.tile_pool(name="tpp", bufs=2, space="PSUM") as tpp,
        ):
            ident_f = cpool.tile([P, P], F32, name="ident_f")
            make_identity(nc, ident_f[:])
            ident = cpool.tile([P, P], F32R, name="ident")
            nc.vector.tensor_copy(ident[:], ident_f[:])
            g = cpool.tile([P, 1], F32, name="g")
            nc.sync.dma_start(g[:], gam[:, :])

            q_tiles = {}

            def load_q(b, cts):
                """Allocate + DMA q tiles (ns-major within this group)."""
                tiles = q_tiles.setdefault(b, {})
                for ct in cts:
                    tiles[ct] = qpool.tile([P, HW], F32R, name=f"q{b}_{ct}",
                                           tag="q")
                for ns in range(NS):
                    for ct in cts:
                        nc.sync.dma_start(
                            tiles[ct][:, ns * 512:(ns + 1) * 512],
                            x[b, ct * P:(ct + 1) * P, ns * 512:(ns + 1) * 512],
                        )

            load_q(0, range(CT))

            for b in range(bpc):
                q = [q_tiles[b][ct] for ct in range(CT)]

                # prefetch half the next batch early; the rest mid-out-phase
                # (keeps the DMA fabric evenly loaded across the batch)
                if b + 1 < bpc:
                    load_q(b + 1, range(2))

                # ---- qT via PE transposes, interleaved with the Gram
                # accumulation. qt tile k ([128, 1024]) packs transpose
                # chunks 2k (cols 0:512) and 2k+1 (cols 512:1024); chunk n
                # holds q[:, n*128:(n+1)*128].T as [n-in-chunk, c].
                qt = [None] * NQT

                def emit_transpose(k, b=b, q=q, qt=qt):
                    tp = tpp.tile([P, 1024], F32, name=f"tp{b}_{k}", tag="tp")
                    for u in range(2):
                        n = 2 * k + u
                        for ct in range(CT):
                            dst = tp[:, u * 512 + ct * P: u * 512 + (ct + 1) * P]
                            nc.tensor.transpose(
                                dst.bitcast(F32R),
                                q[ct][:, n * P:(n + 1) * P],
                                ident[:],
                            )
                    qtk = qtpool.tile([P, 1024], F32R, name=f"qt{b}_{k}", tag="qt")
                    if k % 2 == 0:
                        nc.vector.tensor_copy(qtk[:], tp[:])
                    else:
                        nc.scalar.copy(qtk[:], tp[:])
                    qt[k] = qtk

                e = [
                    mmp.tile([P, C], F32, name=f"e{b}_{i}", tag="mm")
                    for i in range(CT)
                ]

                def emit_energy(k, b=b, e=e, qt=qt):
                    for u in range(2):
                        first = (k == 0 and u == 0)
                        last = (k == NQT - 1 and u == 1)
                        rhs = qt[k][:, u * 512:(u + 1) * 512]
                        for i in range(CT):
                            lhsT = qt[k][:, u * 512 + i * P: u * 512 + (i + 1) * P]
                            nc.tensor.matmul(
                                e[i][:], lhsT=lhsT, rhs=rhs,
                                start=first, stop=last,
                            )

                emit_transpose(0)
                emit_transpose(1)
                for k in range(NQT):
                    if k + 2 < NQT:
                        emit_transpose(k + 2)
                    emit_energy(k)

                # ---- softmax chains for all row tiles first (keeps the
                # in-order DVE/ACT queues from serializing softmax(i+1)
                # behind tile i's residual adds)
                fs, atTs = [], []
                for i in range(CT):
                    mn = spool.tile([P, 1], F32, name=f"mn{b}_{i}", tag="mn")
                    nc.vector.tensor_reduce(
                        mn[:], e[i][:], axis=AX.X, op=OPT.min
                    )
                    # ea = exp(mn - e) (unnormalized attention row block,
                    # rounded to f32r for the PE), s = row sums
                    ea = apool.tile([P, C], F32R, name=f"att{b}_{i}", tag="att")
                    s = spool.tile([P, 1], F32, name=f"s{b}_{i}", tag="s")
                    nc.scalar.activation(
                        ea[:], e[i][:], ACTF.Exp,
                        bias=mn[:], scale=-1.0, accum_out=s[:],
                    )
                    # f = gamma / s  (folded into the residual add below)
                    rinv = spool.tile([P, 1], F32, name=f"ri{b}_{i}", tag="ri")
                    nc.vector.reciprocal(rinv[:], s[:])
                    f = spool.tile([P, 1], F32, name=f"f{b}_{i}", tag="f")
                    nc.vector.tensor_mul(f[:], rinv[:], g[:])
                    fs.append(f)

                    # attT_i[:, j*128:(j+1)*128] = ea[:, j-block].T
                    tq = tpp.tile([P, 512], F32, name=f"tq{b}_{i}", tag="tp")
                    for j in range(CT):
                        nc.tensor.transpose(
                            tq[:, j * P:(j + 1) * P].bitcast(F32R),
                            ea[:, j * P:(j + 1) * P],
                            ident[:],
                        )
                    atT = atpool.tile([P, C], F32R, name=f"attT{b}_{i}", tag="attT")
                    if i % 2 == 0:
                        nc.scalar.copy(atT[:], tq[:])
                    else:
                        nc.vector.tensor_copy(atT[:], tq[:])
                    atTs.append(atT)

                # ---- out rows: ot = (attT_i.T @ q) * f + x, in 16 groups
                # of [128, 1024]. x-residual DMAs are issued ~6 groups ahead
                # so the fp32 x data is resident when the adds run.
                groups = [(i, h) for i in range(CT) for h in range(4)]
                ots = {}

                def alloc_ot(i, h, b=b, ots=ots):
                    ot = opool.tile([P, 1024], F32, name=f"ot{b}_{i}_{h}",
                                    tag="ot")
                    if exact:
                        nc.sync.dma_start(
                            ot[:],
                            x[b, i * P:(i + 1) * P, h * 1024:(h + 1) * 1024]
                            .bitcast(F32),
                        )
                    ots[(i, h)] = ot

                LOOKAHEAD = 6
                for idx in range(LOOKAHEAD):
                    alloc_ot(*groups[idx])
                for idx, (i, h) in enumerate(groups):
                    if idx + LOOKAHEAD < len(groups):
                        alloc_ot(*groups[idx + LOOKAHEAD])
                    if idx == 8 and b + 1 < bpc:
                        load_q(b + 1, range(2, CT))
                    ot = ots.pop((i, h))
                    f, atT = fs[i], atTs[i]
                    for v in range(2):
                        nn = h * 2 + v
                        op = mmp.tile([P, 512], F32, name=f"op{b}_{i}_{nn}",
                                      tag="mm")
                        for j in range(CT):
                            nc.tensor.matmul(
                                op[:],
                                lhsT=atT[:, j * P:(j + 1) * P],
                                rhs=q[j][:, nn * 512:(nn + 1) * 512],
                                start=(j == 0), stop=(j == CT - 1),
                            )
                        if exact:
                            nc.vector.scalar_tensor_tensor(
                                ot[:, v * 512:(v + 1) * 512],
                                op[:],
                                f[:],
                                ot[:, v * 512:(v + 1) * 512],
                                op0=OPT.mult,
                                op1=OPT.add,
                            )
                        else:
                            nc.vector.scalar_tensor_tensor(
                                ot[:, v * 512:(v + 1) * 512],
                                op[:],
                                f[:],
                                q[i][:, nn * 512:(nn + 1) * 512].bitcast(F32),
                                op0=OPT.mult,
                                op1=OPT.add,
                            )
                    nc.sync.dma_start(
                        out[b, i * P:(i + 1) * P, h * 1024:(h + 1) * 1024],
                        ot[:],
                    )

    nc.compile()
    return nc


def build_v2(bpc=BPC):
    """v2: single fp32 x load per batch (exact residual from SBUF), fp32
    PE transposes, f32r produced only by on-chip rounding copies; the out
    matmul rhs comes from a small round-copy ring (out loop h-major)."""
    nc = bacc.Bacc(None, target_bir_lowering=False, debug=False)
    x = nc.dram_tensor("x", [bpc, C, HW], F32, kind="ExternalInput")
    gam = nc.dram_tensor("gamma", [P, 1], F32, kind="ExternalInput")
    out = nc.dram_tensor("out", [bpc, C, HW], F32, kind="ExternalOutput")

    with TileContext(nc) as tc:
        with (
            tc.tile_pool(name="cpool", bufs=1) as cpool,
            tc.tile_pool(name="xfpool", bufs=6) as xfpool,
            tc.tile_pool(name="qtpool", bufs=6) as qtpool,
            tc.tile_pool(name="qrpool", bufs=8) as qrpool,
            tc.tile_pool(name="apool", bufs=4) as apool,
            tc.tile_pool(name="atpool", bufs=4) as atpool,
            tc.tile_pool(name="opool", bufs=4) as opool,
            tc.tile_pool(name="spool", bufs=6) as spool,
            tc.tile_pool(name="mmp", bufs=4, space="PSUM") as mmp,
            tc.tile_pool(name="tpp", bufs=2, space="PSUM") as tpp,
        ):
            ident_f = cpool.tile([P, P], F32, name="ident_f")
            make_identity(nc, ident_f[:])
            ident = cpool.tile([P, P], F32R, name="ident")
            nc.vector.tensor_copy(ident[:], ident_f[:])
            g = cpool.tile([P, 1], F32, name="g")
            nc.sync.dma_start(g[:], gam[:, :])

            xf_tiles = {}

            def load_xf(b, cts, fine_first=False):
                tiles = xf_tiles.setdefault(b, {})
                for ct in cts:
                    tiles[ct] = xfpool.tile([P, HW], F32, name=f"xf{b}_{ct}",
                                            tag="xf")
                if fine_first:
                    # first transpose chunk needs only cols 0:128 of each
                    # tile: land those first so the PE starts sooner
                    for ct in cts:
                        nc.sync.dma_start(
                            tiles[ct][:, 0:P],
                            x[b, ct * P:(ct + 1) * P, 0:P],
                        )
                for ns in range(NS):
                    for ct in cts:
                        lo = P if (fine_first and ns == 0) else 0
                        nc.sync.dma_start(
                            tiles[ct][:, ns * 512 + lo:(ns + 1) * 512],
                            x[b, ct * P:(ct + 1) * P, ns * 512 + lo:(ns + 1) * 512],
                        )

            load_xf(0, range(CT), fine_first=True)

            for b in range(bpc):
                xf = [xf_tiles[b][ct] for ct in range(CT)]

                if b + 1 < bpc:
                    load_xf(b + 1, range(2))

                # ---- f32r slice ring (qs): rounded copies of xf used as
                # transpose inputs (f32r transposes are 1.5 cyc/row vs 2.0
                # for fp32); the same pool serves the out-phase rhs ring.
                qs = {}

                def load_qs(h, b=b, xf=xf, qs=qs):
                    for ct in range(CT):
                        t = qrpool.tile([P, 1024], F32R, name=f"qs{b}_{h}_{ct}",
                                        tag="qr")
                        src = xf[ct][:, h * 1024:(h + 1) * 1024]
                        if (h + ct) % 2 == 0:
                            nc.vector.tensor_copy(t[:, 0:512], src[:, 0:512])
                            nc.scalar.copy(t[:, 512:1024], src[:, 512:1024])
                        else:
                            nc.scalar.copy(t[:, 0:512], src[:, 0:512])
                            nc.vector.tensor_copy(t[:, 512:1024], src[:, 512:1024])
                        qs[(h, ct)] = t

                # ---- qT via f32r PE transposes; Gram accumulation in f32r
                qt = [None] * NQT

                def emit_transpose(k, b=b, xf=xf, qs=qs, qt=qt):
                    tp = tpp.tile([P, 1024], F32, name=f"tp{b}_{k}", tag="tp")
                    for u in range(2):
                        n = 2 * k + u
                        h, r = n // 8, n % 8
                        for ct in range(CT):
                            dst = tp[:, u * 512 + ct * P: u * 512 + (ct + 1) * P]
                            if h == 0:
                                # first column group straight from xf (fp32
                                # transpose): avoids the DMA->round-copy
                                # latency chain at batch start
                                nc.tensor.transpose(
                                    dst,
                                    xf[ct][:, n * P:(n + 1) * P],
                                    ident_f[:],
                                )
                            else:
                                nc.tensor.transpose(
                                    dst.bitcast(F32R),
                                    qs[(h, ct)][:, r * P:(r + 1) * P],
                                    ident[:],
                                )
                    qtk = qtpool.tile([P, 1024], F32R, name=f"qt{b}_{k}", tag="qt")
                    # split the PSUM->SBUF round-copy across both engines so
                    # the tp slot frees at half-copy latency
                    nc.vector.tensor_copy(qtk[:, 0:512], tp[:, 0:512])
                    nc.scalar.copy(qtk[:, 512:1024], tp[:, 512:1024])
                    qt[k] = qtk

                e = [
                    mmp.tile([P, C], F32, name=f"e{b}_{i}", tag="mm")
                    for i in range(CT)
                ]

                # energy rows only from column ECOL[i] on (the Gram matrix
                # is symmetric; a 128-wide f32r matmul runs at 1/4 rate, so
                # row 3 starts at 256 not 384). The missing lower blocks are
                # mirrored from their transposes afterwards.
                ECOL = [0, P, 2 * P, 2 * P]

                def emit_energy(k, b=b, e=e, qt=qt):
                    for u in range(2):
                        first = (k == 0 and u == 0)
                        last = (k == NQT - 1 and u == 1)
                        for i in (0, 1, 2, 3):
                            c0 = ECOL[i]
                            rhs = qt[k][:, u * 512 + c0:(u + 1) * 512]
                            lhsT = qt[k][:, u * 512 + i * P: u * 512 + (i + 1) * P]
                            nc.tensor.matmul(
                                e[i][:, c0:], lhsT=lhsT, rhs=rhs,
                                start=first, stop=last,
                            )

                load_qs(1)
                emit_transpose(0)
                emit_transpose(1)
                for k in range(NQT):
                    if k % 4 == 0 and (k // 4 + 2) < 4:
                        load_qs(k // 4 + 2)
                    if k + 2 < NQT:
                        emit_transpose(k + 2)
                    emit_energy(k)

                # ---- softmax chains for all row tiles; row 3 first (it
                # needs no mirrored blocks, so its chain starts immediately
                # after the last Gram matmul)
                fs, atTs = {}, {}

                def emit_mirrors(b=b, e=e):
                    mst = apool.tile([P, 5 * P], F32R, name=f"mst{b}", tag="mst")
                    for m, (di, dj) in enumerate(
                            [(1, 0), (2, 0), (2, 1), (3, 0), (3, 1)]):
                        nc.scalar.copy(
                            mst[:, m * P:(m + 1) * P],
                            e[dj][:, di * P:(di + 1) * P],
                        )
                        nc.tensor.transpose(
                            e[di][:, dj * P:(dj + 1) * P].bitcast(F32R),
                            mst[:, m * P:(m + 1) * P],
                            ident[:],
                        )

                for i in (0, 1, 2, 3):
                    mn = spool.tile([P, 1], F32, name=f"mn{b}_{i}", tag="mn")
                    nc.vector.tensor_reduce(
                        mn[:], e[i][:], axis=AX.X, op=OPT.min
                    )
                    ea = apool.tile([P, C], F32R, name=f"att{b}_{i}", tag="att")
                    s = spool.tile([P, 1], F32, name=f"s{b}_{i}", tag="s")
                    nc.scalar.activation(
                        ea[:], e[i][:], ACTF.Exp,
                        bias=mn[:], scale=-1.0, accum_out=s[:],
                    )
                    rinv = spool.tile([P, 1], F32, name=f"ri{b}_{i}", tag="ri")
                    nc.vector.reciprocal(rinv[:], s[:])
                    f = spool.tile([P, 1], F32, name=f"f{b}_{i}", tag="f")
                    nc.vector.tensor_mul(f[:], rinv[:], g[:])
                    fs[i] = f

                    tq = tpp.tile([P, 512], F32, name=f"tq{b}_{i}", tag="tp")
                    for j in range(CT):
                        nc.tensor.transpose(
                            tq[:, j * P:(j + 1) * P].bitcast(F32R),
                            ea[:, j * P:(j + 1) * P],
                            ident[:],
                        )
                    atT = atpool.tile([P, C], F32R, name=f"attT{b}_{i}", tag="attT")
                    if i % 2 == 0:
                        nc.scalar.copy(atT[:], tq[:])
                    else:
                        nc.vector.tensor_copy(atT[:], tq[:])
                    atTs[i] = atT
                    if i == 0:
                        emit_mirrors()

                # ---- out phase, h-major: for each column group h (1024 wide)
                # round-copy the 4 rhs slices xf[j][:, h] -> f32r ring, then
                # compute the 4 row tiles i.
                qr = {}

                def load_qr(h, b=b, xf=xf, qr=qr):
                    for j in range(CT):
                        t = qrpool.tile([P, 1024], F32R, name=f"qr{b}_{h}_{j}",
                                        tag="qr")
                        src = xf[j][:, h * 1024:(h + 1) * 1024]
                        if j % 2 == 0:
                            nc.vector.tensor_copy(t[:, 0:512], src[:, 0:512])
                            nc.scalar.copy(t[:, 512:1024], src[:, 512:1024])
                        else:
                            nc.scalar.copy(t[:, 0:512], src[:, 0:512])
                            nc.vector.tensor_copy(t[:, 512:1024], src[:, 512:1024])
                        qr[(h, j)] = t

                load_qr(0)
                for h in range(4):
                    if h + 1 < 4:
                        load_qr(h + 1)
                    if h == 2 and b + 1 < bpc:
                        load_xf(b + 1, range(2, CT))
                    for i in (0, 1, 2, 3):
                        f, atT = fs[i], atTs[i]
                        ot = opool.tile([P, 1024], F32, name=f"ot{b}_{i}_{h}",
                                        tag="ot")
                        for v in range(2):
                            nn = h * 2 + v
                            op = mmp.tile([P, 512], F32, name=f"op{b}_{i}_{nn}",
                                          tag="mm")
                            for j in range(CT):
                                nc.tensor.matmul(
                                    op[:],
                                    lhsT=atT[:, j * P:(j + 1) * P],
                                    rhs=qr[(h, j)][:, v * 512:(v + 1) * 512],
                                    start=(j == 0), stop=(j == CT - 1),
                                )
                            nc.vector.scalar_tensor_tensor(
                                ot[:, v * 512:(v + 1) * 512],
                                op[:],
                                f[:],
                                xf[i][:, nn * 512:(nn + 1) * 512],
                                op0=OPT.mult,
                                op1=OPT.add,
                            )
                        nc.sync.dma_start(
                            out[b, i * P:(i + 1) * P, h * 1024:(h + 1) * 1024],
                            ot[:],
                        )
                    for j in range(CT):
                        qr.pop((h, j))
    nc.compile()
    return nc


_NC_CACHE = None


def _get_nc():
    global _NC_CACHE
    if _NC_CACHE is None:
        _NC_CACHE = build_v2()
    return _NC_CACHE


def run(x, gamma, trace=False):
    """x: [16, 512, 64, 64] fp32; gamma: [1] fp32. Returns (y, results)."""
    x = np.ascontiguousarray(np.asarray(x, dtype=np.float32)).reshape(B, C, HW)
    gval = np.float32(np.asarray(gamma, dtype=np.float32).reshape(-1)[0])
    gbc = np.full((P, 1), gval, dtype=np.float32)
    nc = _get_nc()
    in_maps = [
        {"x": np.ascontiguousarray(x[i * BPC:(i + 1) * BPC]), "gamma": gbc}
        for i in range(NCORES)
    ]
    results = run_bass_kernel_spmd(
        nc, in_maps, core_ids=list(range(NCORES)), trace=trace
    )
    y = np.concatenate([r["out"] for r in results.results], axis=0)
    return y.reshape(B, C, 64, 64), results


def kernel(x, gamma):
    y, _ = run(x, gamma)
    return y


if __name__ == "__main__":
    rng = np.random.default_rng(0)
    x = rng.standard_normal((B, C, 64, 64)).astype(np.float32)
    gamma = np.zeros((1,), dtype=np.float32)
    y, _ = run(x, gamma)
    print("gamma=0 exact:", np.array_equal(y, x))
